# revision 15
# baseline (speedup 1.0000x reference)
"""Trainium2 Bass kernel for masked two-template sparse attention.

Model (per sample, fp32 reference):
    qkv = (x @ W_qkv.T) * mask          mask: temp_mask on first 64 tokens, 1 elsewhere
    q,k,v split into 12 heads x 64
    template tokens (first 128) attend to template tokens only
    search tokens (last 324) attend to all 452 tokens
    out = concat(attn outputs) @ W_proj.T + b_proj

Sharding: data-parallel over batch, 32 samples -> 4 per NeuronCore x 8 cores.
All attention math is done in "transposed" layout (channels on partitions):
    x^T (PE transpose) -> q^T,k^T = Wqkv^T.T @ x^T ; v natural = x^T.T @ Wv^T
    S^T = k^T.T @ q^T  (row-tiled pairs of 64-wide heads run concurrently)
    E^T = exp(S^T * scale)             (no max subtraction; |S| <~ 6)
    [attn^T_unnorm ; sums] = [v_h|ones].T @ E^T  (one matmul per head/k-chunk;
                              partitions 0:64 = attn.V, 64:128 = denominator)
    attn^T = attn^T_unnorm * recip(sums)         (partition-shifted DVE ops)
    y = attn^T.T @ Wp^T                (+ bias added on host)
"""

import numpy as np
import ml_dtypes

import concourse.bass as bass
import concourse.mybir as mybir
import concourse.tile as tile
from concourse.bass_utils import run_bass_kernel_spmd
from concourse.masks import make_identity

# ---------------- configuration ----------------
# dtype domains:
#   PROJ_DT: qkv + output projection operands (weights, x^T, attn^T).
#            float32r = fp32 storage, PE reduced-precision single-pass mode
#            (1 cycle/row at N>=256, vs 4 for float32).
#   ATT_DT:  attention operands (q^T,k^T,v,E,ones). bfloat16 allows the
#            col-tiled head-pair matmuls (float32r does not).
PROJ_DT_NAME = "bfloat16"
ATT_DT_NAME = "bfloat16"
TRACE = False        # request NTFF profile on run
PHASES = 99          # debug: 1=load/xT 2=+qkv 3=+scores/exp 4=+PV/sums 5=+norm 6=full
REPS = 1             # timing: repeat the whole computation inside the NEFF

NCORES = 8
S = 4                # samples per core
N, C, H, DH = 452, 768, 12, 64
NMT, NS = 128, 324   # template tokens / search tokens
SCALE = DH ** -0.5
TCH = [(0, 128), (128, 256), (256, 384), (384, 452)]  # token chunks
KC = 6               # channel chunks of 128

_F32 = mybir.dt.float32


def _legalize_waits(nc, max_waits=1):
    """This container's walrus accepts at most one sync-wait per instruction;
    hoist extra waits onto dedicated NOPs in front of the instruction."""
    n_split = 0
    for f in nc.m.functions:
        for bb in f.blocks:
            new_insts = []
            for inst in bb.instructions:
                si = inst.sync_info
                if si is not None and si.on_wait and len(si.on_wait) > max_waits:
                    waits = list(si.on_wait)
                    for i, w in enumerate(waits[:-max_waits]):
                        new_insts.append(
                            mybir.InstNoOp(
                                name=f"{inst.name}-w{i}",
                                sync_info=mybir.SyncInfo(on_wait=[w], on_update=[]),
                                bass_nofuse=True,
                                engine=inst.engine,
                            )
                        )
                    si.on_wait = waits[-max_waits:]
                    n_split += 1
                new_insts.append(inst)
            bb.instructions = new_insts
    return n_split


def build_module():
    pdt = getattr(mybir.dt, PROJ_DT_NAME)
    adt = getattr(mybir.dt, ATT_DT_NAME)
    # float32r cannot target PSUM partition offsets (no col-tiling)
    assert not (ATT_DT_NAME == "float32r")

    def mm(ap):
        return ap

    # cross-sample double buffering where SBUF allows (fp32 config is tight)
    BUF = 1 if (PROJ_DT_NAME != "bfloat16" and ATT_DT_NAME != "bfloat16") else 2

    nc = bass.Bass("TRN2", target_bir_lowering=False, debug=False)
    x_d = nc.dram_tensor("x", [S, N, C], _F32, kind="ExternalInput").ap()
    m_d = nc.dram_tensor("tmask", [S, 64], _F32, kind="ExternalInput").ap()
    wq_d = nc.dram_tensor("wqkvT", [C, 3 * C], pdt, kind="ExternalInput").ap()
    wp_d = nc.dram_tensor("wpT", [C, C], pdt, kind="ExternalInput").ap()
    y_d = nc.dram_tensor("y", [S, N, C], _F32, kind="ExternalOutput").ap()

    Exp = mybir.ActivationFunctionType.Exp

    with tile.TileContext(nc) as tc:
        with (
            tc.tile_pool(name="const", bufs=1) as cp,
            tc.tile_pool(name="work", bufs=1) as wk,
            tc.tile_pool(name="pa", bufs=2, space="PSUM") as pa,
            tc.tile_pool(name="pb", bufs=1, space="PSUM") as pb,
        ):
            # ---- persistent constants ----
            wq_sb = []
            for i in range(KC):
                w = cp.tile([128, 3 * C], pdt, name=f"wq{i}", tag=f"wq{i}")
                nc.scalar.dma_start(w[:, :], wq_d[i * 128:(i + 1) * 128, :])
                wq_sb.append(w)
            wp_sb = []
            for i in range(KC):
                w = cp.tile([128, C], pdt, name=f"wp{i}", tag=f"wp{i}")
                nc.scalar.dma_start(w[:, :], wp_d[i * 128:(i + 1) * 128, :])
                wp_sb.append(w)
            ident = cp.tile([128, 128], adt, name="ident", tag="ident")
            make_identity(nc, ident)

            def load_dma(s):
                """DMA sample s's x and mask into fresh tiles (no DVE work)."""
                xn = wk.tile([128, 4, C], _F32, name="xn", tag="xn", bufs=2)
                # Pool-engine DMA queue: keeps the big x prefetch off the
                # sync queue that carries the y output stores
                nc.gpsimd.dma_start(
                    xn[:, 0:3, :],
                    x_d[s, 0:384, :].rearrange("(c p) d -> p c d", p=128),
                )
                nc.gpsimd.dma_start(xn[0:68, 3, :], x_d[s, 384:452, :])
                msk = wk.tile([64, 1], _F32, name="msk", tag="msk", bufs=2)
                nc.gpsimd.dma_start(msk[:, :], m_d[s, :].unsqueeze(1))
                if ATT_DT_NAME != "bfloat16":
                    return (xn, msk, xn)
                xnc = wk.tile([128, 4, C], adt, name="xnc", tag="xnc", bufs=2)
                return (xn, msk, xnc)

            def cast_piece(st, i):
                """i-th DVE op of the mask+bf16-cast for a prefetched sample.
                Emitted spread across the previous sample's attention pairs
                (software pipelining): the in-order DVE queue digests them in
                its idle slots, so the next sample's PE transposes never wait
                at the sample boundary — a boundary stall can cross the HAM
                MID window and re-throttle the PE clock to 1.2 GHz."""
                xn, msk, xnc = st
                if ATT_DT_NAME != "bfloat16":
                    if i == 0:
                        nc.vector.tensor_scalar_mul(
                            xn[0:64, 0, :], xn[0:64, 0, :], msk[0:64, :]
                        )
                    return
                if i == 0:
                    # mask fused into the chunk-0 cast (bf16 write applies it)
                    nc.vector.tensor_scalar_mul(
                        xnc[0:64, 0, :], xn[0:64, 0, :], msk[0:64, :]
                    )
                elif i == 1:
                    nc.vector.tensor_copy(xnc[64:128, 0, :], xn[64:128, 0, :])
                else:
                    nc.vector.tensor_copy(xnc[:, i - 1, :], xn[:, i - 1, :])

            N_PIECES = 5
            sched = [si for _rep in range(REPS) for si in range(S)]
            pending = load_dma(sched[0])
            for i in range(N_PIECES):
                cast_piece(pending, i)
            for it, s in enumerate(sched):
                xnc = pending[2]

                # ---- x^T via PE transpose ----
                xTb = wk.tile([128, KC, N], pdt, name="xTb", tag="xTb", bufs=2)
                xT = [xTb[:, cc, :] for cc in range(KC)]
                for ti, (t0, t1) in enumerate(TCH):
                    tsz = t1 - t0
                    if ATT_DT_NAME == "bfloat16":
                        ptr = pa.tile([128, 2048], adt, name="pa_tr", tag="pa")
                    else:
                        ptr = pa.tile([128, 1024], _F32, name="pa_tr", tag="pa")
                    for cc in range(KC):
                        nc.tensor.transpose(
                            ptr[:, cc * 128: cc * 128 + tsz],
                            xnc[0:tsz, ti, cc * 128:(cc + 1) * 128],
                            ident[0:tsz, 0:tsz],
                        )
                    nc.vector.tensor_copy(
                        xTb[:, :, t0:t1],
                        ptr.rearrange("p (c k) -> p c k", k=128)[:, 0:KC, 0:tsz],
                    )

                if PHASES < 2:
                    if it + 1 < len(sched):
                        pending = load_dma(sched[it + 1])
                        for i in range(N_PIECES):
                            cast_piece(pending, i)
                    continue
                # ---- q^T / k^T projections (12 chunks of 128 channels) ----
                qkT = []
                for oc in range(12):
                    pq = pa.tile([128, 1024], _F32, name="pa_qk", tag="pa")
                    for kc in range(KC):
                        nc.tensor.matmul(
                            pq[:, 0:N],
                            mm(wq_sb[kc][:, oc * 128:(oc + 1) * 128]),
                            mm(xT[kc]),
                            start=(kc == 0),
                            stop=(kc == KC - 1),
                        )
                    t = wk.tile([128, N], adt, name=f"qkT{oc}", tag=f"qkT{oc}", bufs=BUF)
                    nc.vector.tensor_copy(t[:, :], pq[:, 0:N])
                    qkT.append(t)

                # prefetch next sample's x (DMAs only; casts slotted into the
                # attention pairs below — see cast_piece)
                have_next = it + 1 < len(sched)
                if have_next:
                    pending = load_dma(sched[it + 1])

                # ---- v, augmented per head as [v_h | ones] (tokens, H*128) ----
                # the ones half makes the PV matmul also emit the softmax
                # denominator (broadcast) into partitions 64:128 for free
                vt = []
                for ti, (t0, t1) in enumerate(TCH):
                    tsz = t1 - t0
                    pv = pa.tile([128, 1024], _F32, name="pa_v", tag="pa")
                    for kc in range(KC):
                        nc.tensor.matmul(
                            pv[0:tsz, 0:512],
                            mm(xT[kc][:, t0:t1]),
                            mm(wq_sb[kc][:, 1536:2048]),
                            start=(kc == 0),
                            stop=(kc == KC - 1),
                        )
                        nc.tensor.matmul(
                            pv[0:tsz, 512:768],
                            mm(xT[kc][:, t0:t1]),
                            mm(wq_sb[kc][:, 2048:2304]),
                            start=(kc == 0),
                            stop=(kc == KC - 1),
                        )
                    t = wk.tile([128, H * 128], adt, name=f"v{ti}", tag=f"v{ti}", bufs=BUF)
                    th = t.rearrange("p (h c) -> p h c", c=128)
                    nc.gpsimd.memset(th[:, :, DH:128], 1.0)
                    nc.vector.tensor_copy(
                        th[0:tsz, :, 0:DH],
                        pv[0:tsz, 0:768].rearrange("p (h c) -> p h c", c=DH),
                    )
                    vt.append(t)

                # ---- attention, head pairs ----
                # Per (pair, head): scores for k-chunk0 computed for ALL 452
                # queries in ONE matmul (template queries see only k 0:128, so
                # the template block lands in cols 0:128 and search in
                # 128:452); k-chunks 1..3 cover search queries only.
                # E layout per head (SBUF, bf16): [0:452] chunk0 (all q),
                # [452:776] chunk1, [776:1100] chunk2, [1100:1424] chunk3.
                # PV then needs only 4 matmuls per head: chunk0's rhs covers
                # cols 0:452 (start=True), chunks 1..3 accumulate cols 128:452.
                if PHASES < 3:
                    continue
                ESW = N + 3 * NS  # 1424
                attnT = []
                for p in range(6):
                    if have_next and p < N_PIECES:
                        cast_piece(pending, p)
                    qc, kt = qkT[p], qkT[6 + p]
                    es_pair = []
                    for hh in range(2):
                        b0 = hh * 64
                        es = wk.tile([128, ESW], adt, name="es", tag="es", bufs=3)
                        # chunk0 (+ template) and chunk1 -> 2-bank tile
                        ps_a = pa.tile([128, 1024], _F32, name="pa_s", tag="pa")
                        nc.tensor.matmul(
                            ps_a[0:128, 0:N],
                            mm(kt[b0:b0 + 64, 0:128]),
                            mm(qc[b0:b0 + 64, 0:N]),
                            start=True, stop=True,
                            tile_position=(b0, 0),
                            skip_group_check=True,
                        )
                        nc.tensor.matmul(
                            ps_a[0:128, 512:512 + NS],
                            mm(kt[b0:b0 + 64, 128:256]),
                            mm(qc[b0:b0 + 64, NMT:N]),
                            start=True, stop=True,
                            tile_position=(b0, 0),
                            skip_group_check=True,
                        )
                        # chunks 2 and 3
                        ps_b = pa.tile([128, 1024], _F32, name="pa_s2", tag="pa")
                        nc.tensor.matmul(
                            ps_b[0:128, 0:NS],
                            mm(kt[b0:b0 + 64, 256:384]),
                            mm(qc[b0:b0 + 64, NMT:N]),
                            start=True, stop=True,
                            tile_position=(b0, 0),
                            skip_group_check=True,
                        )
                        nc.tensor.matmul(
                            ps_b[0:68, 512:512 + NS],
                            mm(kt[b0:b0 + 64, 384:452]),
                            mm(qc[b0:b0 + 64, NMT:N]),
                            start=True, stop=True,
                            tile_position=(b0, 0),
                            skip_group_check=True,
                        )
                        if PHASES >= 3.2:
                            nc.scalar.activation(
                                es[:, 0:N], ps_a[:, 0:N], Exp, scale=SCALE
                            )
                            nc.scalar.activation(
                                es[:, N:N + NS], ps_a[:, 512:512 + NS],
                                Exp, scale=SCALE,
                            )
                            nc.scalar.activation(
                                es[:, N + NS:ESW].rearrange(
                                    "p (b k) -> p b k", k=NS
                                ),
                                ps_b.rearrange("p (b k) -> p b k", b=2)[:, :, 0:NS],
                                Exp,
                                scale=SCALE,
                            )
                        es_pair.append(es)

                    if PHASES < 4:
                        continue
                    # PV + denominators: out partitions 0:64 = attn^T unnorm,
                    # 64:128 = sums broadcast (ones trick)
                    at = wk.tile([128, N], pdt, name=f"attnT{p}", tag=f"attnT{p}", bufs=2)
                    for hh in range(2):
                        h = 2 * p + hh
                        es = es_pair[hh]
                        pvps = pb.tile([128, 512], _F32, name="pv", tag="pv", bufs=2)
                        nc.tensor.matmul(
                            pvps[:, 0:N],
                            mm(vt[0][0:128, h * 128:(h + 1) * 128]),
                            mm(es[0:128, 0:N]),
                            start=True, stop=False,
                            skip_group_check=True,
                        )
                        for kcj in range(1, 4):
                            k0, k1 = TCH[kcj]
                            ksz = k1 - k0
                            nc.tensor.matmul(
                                pvps[:, NMT:N],
                                mm(vt[kcj][0:ksz, h * 128:(h + 1) * 128]),
                                mm(es[0:ksz, N + (kcj - 1) * NS:N + kcj * NS]),
                                start=False, stop=(kcj == 3),
                                skip_group_check=True,
                            )
                        if PHASES < 5:
                            continue
                        r = wk.tile([64, N], _F32, name="r", tag="r", bufs=3)
                        nc.vector.reciprocal(r[0:64, :], pvps[64:128, 0:N])
                        nc.vector.tensor_mul(
                            at[hh * 64:(hh + 1) * 64, :], pvps[0:64, 0:N], r[0:64, :]
                        )
                    attnT.append(at)

                if PHASES < 6:
                    continue
                # ---- output projection (bias added on host) ----
                for (q0, q1) in TCH:
                    qsz = q1 - q0
                    py = pa.tile([128, 1024], _F32, name="pa_y", tag="pa")
                    for mc in range(KC):
                        nc.tensor.matmul(
                            py[0:qsz, 0:512],
                            mm(attnT[mc][:, q0:q1]),
                            mm(wp_sb[mc][:, 0:512]),
                            start=(mc == 0), stop=(mc == KC - 1),
                        )
                        nc.tensor.matmul(
                            py[0:qsz, 512:768],
                            mm(attnT[mc][:, q0:q1]),
                            mm(wp_sb[mc][:, 512:768]),
                            start=(mc == 0), stop=(mc == KC - 1),
                        )
                    ysb = wk.tile([128, C], _F32, name="ysb", tag="ysb", bufs=3)
                    nc.scalar.copy(ysb[0:qsz, :], py[0:qsz, 0:768])
                    nc.sync.dma_start(y_d[s, q0:q1, :], ysb[0:qsz, :])

    _legalize_waits(nc)
    return nc


_NC_CACHE = {}


def _get_module():
    key = (PROJ_DT_NAME, ATT_DT_NAME, PHASES, REPS)
    if key not in _NC_CACHE:
        _NC_CACHE[key] = build_module()
    return _NC_CACHE[key]


def kernel(x, temp_mask, W_qkv, W_proj, b_proj, t_h=None, t_w=None, s_h=None, s_w=None):
    x = np.asarray(x, dtype=np.float32)
    temp_mask = np.asarray(temp_mask, dtype=np.float32)
    B = x.shape[0]
    assert x.shape == (32, N, C), x.shape

    pdt_np = ml_dtypes.bfloat16 if PROJ_DT_NAME == "bfloat16" else np.float32
    adt_np = ml_dtypes.bfloat16 if ATT_DT_NAME == "bfloat16" else np.float32
    wqkvT = np.ascontiguousarray(np.asarray(W_qkv, np.float32).T).astype(pdt_np)
    wpT = np.ascontiguousarray(np.asarray(W_proj, np.float32).T).astype(pdt_np)
    tm = np.ascontiguousarray(temp_mask.reshape(B, 64))

    nc = _get_module()
    per = B // NCORES
    in_maps = [
        {
            "x": np.ascontiguousarray(x[c * per:(c + 1) * per]),
            "tmask": np.ascontiguousarray(tm[c * per:(c + 1) * per]),
            "wqkvT": wqkvT,
            "wpT": wpT,
        }
        for c in range(NCORES)
    ]
    res = run_bass_kernel_spmd(nc, in_maps, core_ids=list(range(NCORES)), trace=TRACE)
    kernel.last_result = res
    y = np.concatenate([res.results[c]["y"] for c in range(NCORES)], axis=0)
    y = y + np.asarray(b_proj, np.float32)[None, None, :]
    return y.astype(np.float32)

